# revision 3
# baseline (speedup 1.0000x reference)
"""MoE gate kernel (softmax + top-6 routing) for Trainium2, 8-core SPMD.

v3: chunk-major matmul order (single open PSUM accumulation group) + SWDGE
streaming.

Measured HW behavior this kernel is shaped around (all N=512 fp16 MMs):
- 210 ns/MM only when BOTH the stationary operand and the PSUM bank repeat;
  any stationary switch costs ~+164 ns (foreground-only weight load:
  LDWEIGHTS serializes against the previous MM's stream+drain; walrus runs
  with --enable-ldw-opt=false so the background weight buffer is unused),
  and any per-MM bank switch / interleaved-open accumulation groups cost
  ~+210 ns.  Since this computation has exactly one MM per (k, stationary,
  bank), the best reachable pattern is chunk-major: 64 consecutive MMs into
  ONE bank with the stationary rotating per MM (~374 ns/MM measured).
- One SWDGE (gpsimd) queue of 2-MiB DMAs streams at ~350 GB/s (HWDGE
  queues saturate at ~215 GB/s each); all 24 MiB/iter of input rides
  nc.gpsimd, outputs ride nc.sync.

Math = v1/v2: x = xh + xl/2048 (fp16 + fp8e4m3-of-scaled-residual, ~2^-15
combined), w = wh + wl/2048 (fp16 pair).  Per chunk the bank accumulates
  rows 0:64   main  = wh.T xh            (stationary [wh|wl], moving xh)
  rows 64:128 cross = wl.T xh + wh.T xl  (second pass: 64-col wh
                                          stationary, moving xl)
logits = main + cross/2048; top-6 on logits; exp(l - l_max) normalized.
Indices match the fp32 reference exactly; weights to ~3e-5.
"""

import sys

for _p in ("/root/.axon_site", "/root/.axon_site/_ro/trn_rl_repo",
           "/root/.axon_site/_ro/pypackages", "/opt/trn_rl_repo"):
    if _p not in sys.path:
        sys.path.append(_p)

import numpy as np

N_CORES = 8
N_TOKENS = 16384
HIDDEN = 4096
N_EXPERTS = 64
TOP_K = 6

T_CORE = N_TOKENS // N_CORES          # 2048 tokens per core
CHUNK = 512                           # tokens per PSUM accumulator bank
N_CHUNKS = T_CORE // CHUNK            # 4
KC = HIDDEN // 128                    # 32 k-chunks of 128
KH = 16                               # k-chunks per xh DMA (2 MiB)
GROUPS = CHUNK // 128                 # 4 transpose groups per chunk

SPLIT_SCALE = 2048.0                  # 2^11: lo parts are scaled by this
XH_BUFS = 7                           # xh tiles in flight (2 per chunk)
XL_BUFS = 3

_PROGRAM = None


def _build_program(n_iters: int = 1):
    import concourse.bacc as bacc
    import concourse.tile as tile
    import concourse.mybir as mybir
    import concourse.bass as bass
    from concourse import masks

    f32 = mybir.dt.float32
    f16 = mybir.dt.float16
    f8 = mybir.dt.float8e4
    i32 = mybir.dt.int32
    u32 = mybir.dt.uint32

    nc = bacc.Bacc("TRN2", target_bir_lowering=False, debug=False,
                   num_devices=N_CORES)

    # chunk-major token streams: hi fp16 (2 DMAs/chunk), lo fp8 (1 DMA/chunk)
    xh_h = nc.dram_tensor("xh", [N_CHUNKS, 2, 128, KH, CHUNK], f16,
                          kind="ExternalInput")
    xl_h = nc.dram_tensor("xl", [N_CHUNKS, 128, KC, CHUNK], f8,
                          kind="ExternalInput")
    # wt1[:, k, 0:64] = wh, wt1[:, k, 64:128] = wl (scaled 2^11)
    wt1_h = nc.dram_tensor("wt1", [128, KC, 2 * N_EXPERTS], f16,
                           kind="ExternalInput")
    oi_h = nc.dram_tensor("oidx", [T_CORE, TOP_K], i32, kind="ExternalOutput")
    ow_h = nc.dram_tensor("ow", [T_CORE, TOP_K], f32, kind="ExternalOutput")

    with tile.TileContext(nc) as tc:
        with (
            tc.tile_pool(name="const", bufs=1) as cpool,
            tc.tile_pool(name="xhin", bufs=XH_BUFS) as xhpool,
            tc.tile_pool(name="xlin", bufs=XL_BUFS) as xlpool,
            tc.tile_pool(name="ps_acc", bufs=3, space=bass.MemorySpace.PSUM) as psacc,
            tc.tile_pool(name="ps_w", bufs=1, space=bass.MemorySpace.PSUM) as pswrm,
            tc.tile_pool(name="ps_tr", bufs=3, space=bass.MemorySpace.PSUM) as pstr,
            tc.tile_pool(name="lg", bufs=2) as lgpool,
            tc.tile_pool(name="tk", bufs=4) as tkpool,
            tc.tile_pool(name="ob", bufs=2) as obpool,
        ):
            # Engine warm-up (PE init, ACT exp table, DVE max/max_index
            # ucode) on dummy tiles; overlaps the first x tiles' DMA.
            wrm = cpool.tile([128, 16], f32)
            nc.gpsimd.memset(wrm[:], 0.0)
            wrm_ps = pswrm.tile([16, 16], f32)
            nc.tensor.matmul(wrm_ps[:], wrm[:, 0:16], wrm[:])
            wrm_e = cpool.tile([128, 16], f32)
            nc.scalar.activation(wrm_e[:], wrm[:],
                                 mybir.ActivationFunctionType.Exp)
            wrm_m = cpool.tile([128, 8], f32)
            nc.vector.max(wrm_m[:], wrm[:])
            wrm_i = cpool.tile([128, 8], u32)
            nc.vector.max_index(wrm_i[:], wrm_m[:], wrm[:])

            wt1 = cpool.tile([128, KC, 2 * N_EXPERTS], f16)
            nc.gpsimd.dma_start(wt1[:], wt1_h.ap())
            ident = cpool.tile([128, 128], f32)
            masks.make_identity(nc, ident[:])

            for it in range(n_iters):
                # batched outputs: [p, c, g, e] -> token c*512 + g*128 + p
                i6b = obpool.tile([128, N_CHUNKS, GROUPS, TOP_K], i32,
                                  tag="i6")
                w6b = obpool.tile([128, N_CHUNKS, GROUPS, TOP_K], f32,
                                  tag="w6")

                for c in range(N_CHUNKS):
                    xa = xhpool.tile([128, KH, CHUNK], f16, tag="xh",
                                     name="xa")
                    nc.gpsimd.dma_start(xa[:], xh_h.ap()[c, 0])
                    xl = xlpool.tile([128, KC, CHUNK], f8, tag="xl",
                                     name="xl")
                    nc.gpsimd.dma_start(xl[:], xl_h.ap()[c])
                    xb = xhpool.tile([128, KH, CHUNK], f16, tag="xh",
                                     name="xb")
                    nc.gpsimd.dma_start(xb[:], xh_h.ap()[c, 1])

                    # 64 MMs into ONE bank: rows 0:64 main, 64:128 cross
                    ps = psacc.tile([128, CHUNK], f32, tag="ps", name="ps")
                    for k in range(KC):
                        xt = xa if k < KH else xb
                        nc.tensor.matmul(ps[:], wt1[:, k, :],
                                         xt[:, k % KH, :],
                                         start=(k == 0), stop=False,
                                         skip_group_check=True)
                    for k in range(KC):
                        nc.tensor.matmul(ps[N_EXPERTS:128, :],
                                         wt1[:, k, 0:N_EXPERTS],
                                         xl[:, k, :],
                                         start=False, stop=(k == KC - 1),
                                         skip_group_check=True)

                    # epilogue: PSUM -> SBUF, transpose, combine, topk, exp
                    tl = lgpool.tile([128, CHUNK], f32, tag="tl")
                    nc.scalar.activation(tl[:], ps[:],
                                         mybir.ActivationFunctionType.Copy)
                    ps_t = pstr.tile([128, GROUPS, 128], f32, tag="pst")
                    sct = lgpool.tile([128, GROUPS, 128], f32, tag="sct")
                    sc = lgpool.tile([128, GROUPS, N_EXPERTS], f32, tag="sc")
                    for g in range(GROUPS):
                        nc.tensor.transpose(ps_t[:, g, :],
                                            tl[:, g * 128:(g + 1) * 128],
                                            ident[:])
                    nc.scalar.activation(sct[:], ps_t[:],
                                         mybir.ActivationFunctionType.Copy)
                    nc.vector.scalar_tensor_tensor(
                        sc[:], sct[:, :, N_EXPERTS:2 * N_EXPERTS],
                        1.0 / SPLIT_SCALE, sct[:, :, 0:N_EXPERTS],
                        mybir.AluOpType.mult, mybir.AluOpType.add)
                    l8 = tkpool.tile([128, GROUPS, 8], f32, tag="l8")
                    ix8 = tkpool.tile([128, GROUPS, 8], u32, tag="ix8")
                    for g in range(GROUPS):
                        nc.vector.max(l8[:, g, :], sc[:, g, :])
                        nc.vector.max_index(ix8[:, g, :], l8[:, g, :],
                                            sc[:, g, :])
                    nc.vector.tensor_copy(i6b[:, c], ix8[:, :, 0:TOP_K])
                    negm = tkpool.tile([128, GROUPS], f32, tag="negm")
                    nc.vector.tensor_scalar_mul(negm[:], l8[:, :, 0], -1.0)
                    e6 = tkpool.tile([128, GROUPS, TOP_K], f32, tag="e6")
                    for g in range(GROUPS):
                        nc.scalar.activation(e6[:, g, :], l8[:, g, 0:TOP_K],
                                             mybir.ActivationFunctionType.Exp,
                                             bias=negm[:, g:g + 1])
                    den = tkpool.tile([128, GROUPS], f32, tag="den")
                    nc.vector.reduce_sum(den[:], e6[:],
                                         axis=mybir.AxisListType.X)
                    rec = tkpool.tile([128, GROUPS], f32, tag="rec")
                    nc.vector.reciprocal(rec[:], den[:])
                    nc.vector.tensor_mul(
                        w6b[:, c], e6[:],
                        rec[:].unsqueeze(2).broadcast_to(
                            (128, GROUPS, TOP_K)))

                pat = [[TOP_K, 128], [CHUNK * TOP_K, N_CHUNKS],
                       [128 * TOP_K, GROUPS], [1, TOP_K]]
                nc.sync.dma_start(bass.AP(oi_h, 0, pat), i6b[:])
                nc.sync.dma_start(bass.AP(ow_h, 0, pat), w6b[:])

    nc.compile()
    return nc


def _get_program():
    global _PROGRAM
    if _PROGRAM is None:
        _PROGRAM = _build_program(1)
    return _PROGRAM


def _prep_inputs(hidden_states: np.ndarray, weight: np.ndarray):
    """Build per-core input maps (token-sharded x, replicated weight)."""
    import ml_dtypes

    w = np.ascontiguousarray(weight.astype(np.float32, copy=False))
    wh = w.astype(np.float16)
    wl = ((w - wh.astype(np.float32)) * SPLIT_SCALE).astype(np.float16)
    w1 = np.concatenate([wh, wl], axis=0)                  # [128, HIDDEN]
    # wt[p, k, c] = w1[c, k*128 + p]
    wt1 = np.ascontiguousarray(w1.T.reshape(KC, 128, 2 * N_EXPERTS)
                               .transpose(1, 0, 2))
    in_maps = []
    for cid in range(N_CORES):
        shard = (hidden_states[cid * T_CORE:(cid + 1) * T_CORE]
                 .astype(np.float32, copy=False))
        sh = shard.astype(np.float16)
        sl = (((shard - sh.astype(np.float32)) * SPLIT_SCALE)
              .astype(ml_dtypes.float8_e4m3))
        # xh[c, h, p, kt, t] = sh[c*512 + t, (h*16 + kt)*128 + p]
        xh = np.ascontiguousarray(
            sh.reshape(N_CHUNKS, CHUNK, 2, KH, 128).transpose(0, 2, 4, 3, 1))
        # xl[c, p, k, t] = sl[c*512 + t, k*128 + p]
        xl = np.ascontiguousarray(
            sl.reshape(N_CHUNKS, CHUNK, KC, 128).transpose(0, 3, 2, 1))
        in_maps.append({"xh": xh, "xl": xl, "wt1": wt1})
    return in_maps


def kernel(hidden_states: np.ndarray, weight: np.ndarray):
    from concourse.bass_utils import run_bass_kernel_spmd

    hidden_states = np.asarray(hidden_states)
    weight = np.asarray(weight)
    nc = _get_program()
    in_maps = _prep_inputs(hidden_states, weight)
    res = run_bass_kernel_spmd(nc, in_maps, list(range(N_CORES)),
                               trace=False)
    idx = np.concatenate([res.results[i]["oidx"] for i in range(N_CORES)],
                         axis=0)
    wgt = np.concatenate([res.results[i]["ow"] for i in range(N_CORES)],
                         axis=0)
    return idx.astype(np.int32, copy=False), wgt.astype(np.float32, copy=False)


# revision 5
# speedup vs baseline: 1.4688x; 1.4688x over previous
"""MoE gate kernel (softmax + top-6 routing) for Trainium2, 8-core SPMD.

v3: chunk-major matmul order (single open PSUM accumulation group) + SWDGE
streaming.

Measured HW behavior this kernel is shaped around (all N=512 fp16 MMs):
- 210 ns/MM only when BOTH the stationary operand and the PSUM bank repeat;
  any stationary switch costs ~+164 ns (foreground-only weight load:
  LDWEIGHTS serializes against the previous MM's stream+drain; walrus runs
  with --enable-ldw-opt=false so the background weight buffer is unused),
  and any per-MM bank switch / interleaved-open accumulation groups cost
  ~+210 ns.  Since this computation has exactly one MM per (k, stationary,
  bank), the best reachable pattern is chunk-major: 64 consecutive MMs into
  ONE bank with the stationary rotating per MM (~374 ns/MM measured).
- One SWDGE (gpsimd) queue of 2-MiB DMAs streams at ~350 GB/s (HWDGE
  queues saturate at ~215 GB/s each); all 24 MiB/iter of input rides
  nc.gpsimd, outputs ride nc.sync.

Math = v1/v2: x = xh + xl/2048 (fp16 + fp8e4m3-of-scaled-residual, ~2^-15
combined), w = wh + wl/2048 (fp16 pair).  Per chunk the bank accumulates
  rows 0:64   main  = wh.T xh            (stationary [wh|wl], moving xh)
  rows 64:128 cross = wl.T xh + wh.T xl  (second pass stationary [0|wh],
                                          moving xl, adds +0.0 to rows 0:64)
logits = main + cross/2048; top-6 on logits; exp(l - l_max) normalized.
Indices match the fp32 reference exactly; weights to ~3e-5.
"""

import sys

for _p in ("/root/.axon_site", "/root/.axon_site/_ro/trn_rl_repo",
           "/root/.axon_site/_ro/pypackages", "/opt/trn_rl_repo"):
    if _p not in sys.path:
        sys.path.append(_p)

import numpy as np

N_CORES = 8
N_TOKENS = 16384
HIDDEN = 4096
N_EXPERTS = 64
TOP_K = 6

T_CORE = N_TOKENS // N_CORES          # 2048 tokens per core
CHUNK = 512                           # tokens per PSUM accumulator bank
N_CHUNKS = T_CORE // CHUNK            # 4
KC = HIDDEN // 128                    # 32 k-chunks of 128
KH = 16                               # k-chunks per xh DMA (2 MiB)
GROUPS = CHUNK // 128                 # 4 transpose groups per chunk

SPLIT_SCALE = 2048.0                  # 2^11: lo parts are scaled by this
XH_BUFS = 8                           # xh tiles in flight (2 per chunk)
XL_BUFS = 3

_PROGRAM = None


def _build_program(n_iters: int = 1):
    import concourse.bacc as bacc
    import concourse.tile as tile
    import concourse.mybir as mybir
    import concourse.bass as bass
    from concourse import masks

    f32 = mybir.dt.float32
    f16 = mybir.dt.float16
    f8 = mybir.dt.float8e4
    i32 = mybir.dt.int32
    u32 = mybir.dt.uint32

    nc = bacc.Bacc("TRN2", target_bir_lowering=False, debug=False,
                   num_devices=N_CORES)

    # chunk-major token streams: hi fp16 (2 DMAs/chunk), lo fp8 (1 DMA/chunk)
    xh_h = nc.dram_tensor("xh", [N_CHUNKS, 2, 128, KH, CHUNK], f16,
                          kind="ExternalInput")
    xl_h = nc.dram_tensor("xl", [N_CHUNKS, 128, KC, CHUNK], f8,
                          kind="ExternalInput")
    # wt1[:, k, 0:64] = wh, wt1[:, k, 64:128] = wl (scaled 2^11)
    # wt2[:, k, 0:64] = 0,  wt2[:, k, 64:128] = wh
    wt1_h = nc.dram_tensor("wt1", [128, KC, 2 * N_EXPERTS], f16,
                           kind="ExternalInput")
    wt2_h = nc.dram_tensor("wt2", [128, KC, 2 * N_EXPERTS], f16,
                           kind="ExternalInput")
    oi_h = nc.dram_tensor("oidx", [T_CORE, TOP_K], i32, kind="ExternalOutput")
    ow_h = nc.dram_tensor("ow", [T_CORE, TOP_K], f32, kind="ExternalOutput")

    with tile.TileContext(nc) as tc:
        with (
            tc.tile_pool(name="const", bufs=1) as cpool,
            tc.tile_pool(name="xhin", bufs=XH_BUFS) as xhpool,
            tc.tile_pool(name="xlin", bufs=XL_BUFS) as xlpool,
            tc.tile_pool(name="ps_acc", bufs=3, space=bass.MemorySpace.PSUM) as psacc,
            tc.tile_pool(name="ps_w", bufs=1, space=bass.MemorySpace.PSUM) as pswrm,
            tc.tile_pool(name="ps_tr", bufs=3, space=bass.MemorySpace.PSUM) as pstr,
            tc.tile_pool(name="lg", bufs=2) as lgpool,
            tc.tile_pool(name="tk", bufs=4) as tkpool,
            tc.tile_pool(name="ob", bufs=2) as obpool,
        ):
            # Engine warm-up (PE init, ACT exp table, DVE max/max_index
            # ucode) on dummy tiles; overlaps the first x tiles' DMA.
            wrm = cpool.tile([128, 16], f32)
            nc.gpsimd.memset(wrm[:], 0.0)
            wrm_ps = pswrm.tile([16, 16], f32)
            nc.tensor.matmul(wrm_ps[:], wrm[:, 0:16], wrm[:])
            wrm_e = cpool.tile([128, 16], f32)
            nc.scalar.activation(wrm_e[:], wrm[:],
                                 mybir.ActivationFunctionType.Exp)
            wrm_m = cpool.tile([128, 8], f32)
            nc.vector.max(wrm_m[:], wrm[:])
            wrm_i = cpool.tile([128, 8], u32)
            nc.vector.max_index(wrm_i[:], wrm_m[:], wrm[:])

            wt1 = cpool.tile([128, KC, 2 * N_EXPERTS], f16)
            nc.gpsimd.dma_start(wt1[:], wt1_h.ap())
            wt2 = cpool.tile([128, KC, 2 * N_EXPERTS], f16)
            nc.gpsimd.dma_start(wt2[:], wt2_h.ap())
            ident = cpool.tile([128, 128], f32)
            masks.make_identity(nc, ident[:])

            for it in range(n_iters):
                # batched outputs: [p, c, g, e] -> token c*512 + g*128 + p
                i6b = obpool.tile([128, N_CHUNKS, GROUPS, TOP_K], i32,
                                  tag="i6")
                w6b = obpool.tile([128, N_CHUNKS, GROUPS, TOP_K], f32,
                                  tag="w6")

                for c in range(N_CHUNKS):
                    xa = xhpool.tile([128, KH, CHUNK], f16, tag="xh",
                                     name="xa")
                    nc.gpsimd.dma_start(xa[:], xh_h.ap()[c, 0])
                    xl = xlpool.tile([128, KC, CHUNK], f8, tag="xl",
                                     name="xl")
                    nc.gpsimd.dma_start(xl[:], xl_h.ap()[c])
                    xb = xhpool.tile([128, KH, CHUNK], f16, tag="xh",
                                     name="xb")
                    nc.gpsimd.dma_start(xb[:], xh_h.ap()[c, 1])

                    # 64 MMs into ONE bank: rows 0:64 main, 64:128 cross
                    ps = psacc.tile([128, CHUNK], f32, tag="ps", name="ps")
                    for k in range(KC):
                        xt = xa if k < KH else xb
                        nc.tensor.matmul(ps[:], wt1[:, k, :],
                                         xt[:, k % KH, :],
                                         start=(k == 0), stop=False,
                                         skip_group_check=True)
                    for k in range(KC):
                        nc.tensor.matmul(ps[:], wt2[:, k, :], xl[:, k, :],
                                         start=False, stop=(k == KC - 1),
                                         skip_group_check=True)

                    # epilogue: PSUM -> SBUF, transpose, combine, topk, exp
                    tl = lgpool.tile([128, CHUNK], f32, tag="tl")
                    nc.scalar.activation(tl[:], ps[:],
                                         mybir.ActivationFunctionType.Copy)
                    ps_t = pstr.tile([128, GROUPS, 128], f32, tag="pst")
                    sct = lgpool.tile([128, GROUPS, 128], f32, tag="sct")
                    sc = lgpool.tile([128, GROUPS, N_EXPERTS], f32, tag="sc")
                    for g in range(GROUPS):
                        nc.tensor.transpose(ps_t[:, g, :],
                                            tl[:, g * 128:(g + 1) * 128],
                                            ident[:])
                    nc.scalar.activation(sct[:], ps_t[:],
                                         mybir.ActivationFunctionType.Copy)
                    nc.vector.scalar_tensor_tensor(
                        sc[:], sct[:, :, N_EXPERTS:2 * N_EXPERTS],
                        1.0 / SPLIT_SCALE, sct[:, :, 0:N_EXPERTS],
                        mybir.AluOpType.mult, mybir.AluOpType.add)
                    l8 = tkpool.tile([128, GROUPS, 8], f32, tag="l8")
                    ix8 = tkpool.tile([128, GROUPS, 8], u32, tag="ix8")
                    for g in range(GROUPS):
                        nc.vector.max(l8[:, g, :], sc[:, g, :])
                        nc.vector.max_index(ix8[:, g, :], l8[:, g, :],
                                            sc[:, g, :])
                    nc.vector.tensor_copy(i6b[:, c], ix8[:, :, 0:TOP_K])
                    negm = tkpool.tile([128, GROUPS], f32, tag="negm")
                    nc.vector.tensor_scalar_mul(negm[:], l8[:, :, 0], -1.0)
                    e6 = tkpool.tile([128, GROUPS, TOP_K], f32, tag="e6")
                    for g in range(GROUPS):
                        nc.scalar.activation(e6[:, g, :], l8[:, g, 0:TOP_K],
                                             mybir.ActivationFunctionType.Exp,
                                             bias=negm[:, g:g + 1])
                    den = tkpool.tile([128, GROUPS], f32, tag="den")
                    nc.vector.reduce_sum(den[:], e6[:],
                                         axis=mybir.AxisListType.X)
                    rec = tkpool.tile([128, GROUPS], f32, tag="rec")
                    nc.vector.reciprocal(rec[:], den[:])
                    nc.vector.tensor_mul(
                        w6b[:, c], e6[:],
                        rec[:].unsqueeze(2).broadcast_to(
                            (128, GROUPS, TOP_K)))

                pat = [[TOP_K, 128], [CHUNK * TOP_K, N_CHUNKS],
                       [128 * TOP_K, GROUPS], [1, TOP_K]]
                nc.sync.dma_start(bass.AP(oi_h, 0, pat), i6b[:])
                nc.sync.dma_start(bass.AP(ow_h, 0, pat), w6b[:])

    nc.compile()
    return nc


def _get_program():
    global _PROGRAM
    if _PROGRAM is None:
        _PROGRAM = _build_program(1)
    return _PROGRAM


def _prep_inputs(hidden_states: np.ndarray, weight: np.ndarray):
    """Build per-core input maps (token-sharded x, replicated weight)."""
    import ml_dtypes

    w = np.ascontiguousarray(weight.astype(np.float32, copy=False))
    wh = w.astype(np.float16)
    wl = ((w - wh.astype(np.float32)) * SPLIT_SCALE).astype(np.float16)
    w1 = np.concatenate([wh, wl], axis=0)                  # [128, HIDDEN]
    w2 = np.concatenate([np.zeros_like(wh), wh], axis=0)   # [128, HIDDEN]
    # wt[p, k, c] = wX[c, k*128 + p]
    wt1 = np.ascontiguousarray(w1.T.reshape(KC, 128, 2 * N_EXPERTS)
                               .transpose(1, 0, 2))
    wt2 = np.ascontiguousarray(w2.T.reshape(KC, 128, 2 * N_EXPERTS)
                               .transpose(1, 0, 2))
    in_maps = []
    for cid in range(N_CORES):
        shard = (hidden_states[cid * T_CORE:(cid + 1) * T_CORE]
                 .astype(np.float32, copy=False))
        sh = shard.astype(np.float16)
        sl = (((shard - sh.astype(np.float32)) * SPLIT_SCALE)
              .astype(ml_dtypes.float8_e4m3))
        # xh[c, h, p, kt, t] = sh[c*512 + t, (h*16 + kt)*128 + p]
        xh = np.ascontiguousarray(
            sh.reshape(N_CHUNKS, CHUNK, 2, KH, 128).transpose(0, 2, 4, 3, 1))
        # xl[c, p, k, t] = sl[c*512 + t, k*128 + p]
        xl = np.ascontiguousarray(
            sl.reshape(N_CHUNKS, CHUNK, KC, 128).transpose(0, 3, 2, 1))
        in_maps.append({"xh": xh, "xl": xl, "wt1": wt1, "wt2": wt2})
    return in_maps


def kernel(hidden_states: np.ndarray, weight: np.ndarray):
    from concourse.bass_utils import run_bass_kernel_spmd

    hidden_states = np.asarray(hidden_states)
    weight = np.asarray(weight)
    nc = _get_program()
    in_maps = _prep_inputs(hidden_states, weight)
    res = run_bass_kernel_spmd(nc, in_maps, list(range(N_CORES)),
                               trace=False)
    idx = np.concatenate([res.results[i]["oidx"] for i in range(N_CORES)],
                         axis=0)
    wgt = np.concatenate([res.results[i]["ow"] for i in range(N_CORES)],
                         axis=0)
    return idx.astype(np.int32, copy=False), wgt.astype(np.float32, copy=False)
